# revision 5
# baseline (speedup 1.0000x reference)
"""DetNetV3 pool_prior_features (bilinear grid_sample along lane priors) on 8 trn2 cores.

Strategy (data-parallel over batch, 4 images per core):
- Host: layout-only prep. Features are transposed to NHWC so one 512B DRAM
  element covers both x-taps (x0,x0+1) of all 64 channels. Priors are
  permuted into the two layouts the device needs (gather-index-wrapped and
  column-on-partition). All constant tiles (y-offsets, y-weights, identity)
  are baked host-side; y0/y1/wy are compile-time constants of the module.
- Device: computes x0 = floor(px*199), fx = frac; gathers the 4 taps with
  two dma_gather calls per image half (512B pair elements, y0-row and
  y1-row); applies the 4-tap bilinear weighted sum with tensor_tensor ops
  (weights broadcast over the channel dim with stride-0 APs); transposes
  (cols,ch) -> (ch,cols) on the tensor engine; DMAs to the final
  (B*N, C, S, 1) layout.
"""

import sys

sys.path.insert(0, "/opt/trn_rl_repo")

import numpy as np

import concourse.bass as bass
import concourse.mybir as mybir
from concourse import bacc
from concourse.bass import AP
from concourse.bass_utils import run_bass_kernel_spmd
from concourse.tile import TileContext

F32 = mybir.dt.float32
I16 = mybir.dt.int16

# ---------------------------------------------------------------- constants
B, C, H, W = 32, 64, 80, 200
N, S = 192, 36
NCORES = 8
BL = B // NCORES          # images per core
HW = H * W                # pixels per image
COLS = N * S              # gather columns per image (6912)
NHALF = COLS // 2         # columns per half (3456)
GROUPS = COLS // 128      # 54
GH = GROUPS // 2          # 27 groups per half
JJ = COLS // 16           # idx tile free dim (432)

# y-side constants, computed exactly as the reference does (float32 ops)
_sx = (np.linspace(0.0, 1.0, S, dtype=np.float32) * 71).astype(np.int64)
PRIOR_FEAT_YS = np.ascontiguousarray(
    np.flip(1.0 - _sx.astype(np.float32) / 71)
).astype(np.float32)
_gy = PRIOR_FEAT_YS * np.float32(2.0) - np.float32(1.0)
_iy = (_gy + np.float32(1.0)) * np.float32(0.5) * np.float32(H - 1)
Y0 = np.floor(_iy)
Y1 = Y0 + 1.0
WY1 = (_iy - Y0) * (Y1 <= H - 1)          # mask: zero weight when y1 off-grid
WY0 = 1.0 - (_iy - Y0)
# reference masks the y0 term too (y0 always in [0,79] here, so m=1)
Y0I = Y0.astype(np.int64)
Y1I = np.minimum(Y1, H - 1).astype(np.int64)
WY0 = WY0.astype(np.float32)
WY1 = WY1.astype(np.float32)

# column -> (n, s): col = n*S + s
_cols = np.arange(COLS)
_s_of_col = (_cols % S).astype(np.int64)

# wrapped idx layout: position (q, jj) holds column jj*16 + (q % 16)
_q = np.arange(128)[:, None]
_jj = np.arange(JJ)[None, :]
COLMAP_W = (_jj * 16 + (_q % 16))          # (128, 432)
# column-on-partition layout: position (p, g) holds column g*128 + p
_p = np.arange(128)[:, None]
_g = np.arange(GROUPS)[None, :]
COLMAP_C = (_g * 128 + _p)                 # (128, 54)

YOFF0 = (Y0I[_s_of_col] * W)[COLMAP_W].astype(np.int16)   # (128, 432)
YOFF1 = (Y1I[_s_of_col] * W)[COLMAP_W].astype(np.int16)
WY0C = WY0[_s_of_col][COLMAP_C].astype(np.float32)        # (128, 54)
WY1C = WY1[_s_of_col][COLMAP_C].astype(np.float32)
IDENT = np.eye(128, dtype=np.float32)

TBL_LEN = BL * HW * C + C   # +1 padded pixel so the last x-pair AP stays in-bounds

_nc_cache = {}


def _build_nc():
    if "nc" in _nc_cache:
        return _nc_cache["nc"]
    nc = bacc.Bacc("TRN2")
    table = nc.dram_tensor("table", [TBL_LEN], F32, kind="ExternalInput")
    pxw = nc.dram_tensor("pxw", [BL, 128, JJ], F32, kind="ExternalInput")
    pxc = nc.dram_tensor("pxc", [BL, 128, GROUPS], F32, kind="ExternalInput")
    yoff0 = nc.dram_tensor("yoff0", [128, JJ], I16, kind="ExternalInput")
    yoff1 = nc.dram_tensor("yoff1", [128, JJ], I16, kind="ExternalInput")
    wy0 = nc.dram_tensor("wy0", [128, GROUPS], F32, kind="ExternalInput")
    wy1 = nc.dram_tensor("wy1", [128, GROUPS], F32, kind="ExternalInput")
    ident = nc.dram_tensor("ident", [128, 128], F32, kind="ExternalInput")
    out = nc.dram_tensor("out", [BL * N * C * S], F32, kind="ExternalOutput")

    with TileContext(nc) as tc:
        with (
            tc.tile_pool(name="const", bufs=1) as cpool,
            tc.tile_pool(name="px", bufs=2) as pxpool,
            tc.tile_pool(name="idx", bufs=2) as idxpool,
            tc.tile_pool(name="gath", bufs=2) as gpool,
            tc.tile_pool(name="lerp", bufs=2) as lpool,
            tc.tile_pool(name="outp", bufs=2) as opool,
            tc.tile_pool(name="psum", bufs=4, space="PSUM") as pspool,
        ):
            yoff0_t = cpool.tile([128, JJ], I16, tag="c0")
            yoff1_t = cpool.tile([128, JJ], I16, tag="c1")
            wy0_t = cpool.tile([128, GROUPS], F32, tag="c2")
            wy1_t = cpool.tile([128, GROUPS], F32, tag="c3")
            ident_t = cpool.tile([128, 128], F32, tag="c4")
            nc.sync.dma_start(yoff0_t[:], yoff0[:])
            nc.sync.dma_start(yoff1_t[:], yoff1[:])
            nc.sync.dma_start(wy0_t[:], wy0[:])
            nc.sync.dma_start(wy1_t[:], wy1[:])
            nc.sync.dma_start(ident_t[:], ident[:])

            for b in range(BL):
                pxw_t = pxpool.tile([128, JJ], F32, tag="pxw")
                pxc_t = pxpool.tile([128, GROUPS], F32, tag="pxc")
                nc.sync.dma_start(pxw_t[:], pxw[b])
                nc.sync.dma_start(pxc_t[:], pxc[b])

                # gather indices (wrapped layout): x0 + y*W.
                # ix matches the reference bit-exactly: gx = px*2-1;
                # ix = (gx+1)*0.5*199 == (gx+1)*99.5 (same single rounding).
                # HW f32->int cast is round-half-even, so cast(ix-0.5) is
                # floor(ix) except at odd integers where it yields k-1 with
                # fx=1 -- the lerp result is identical either way.
                ixw = idxpool.tile([128, JJ], F32, tag="ixw")
                x0w = idxpool.tile([128, JJ], F32, tag="x0w")
                x0i = idxpool.tile([128, JJ], I16, tag="x0i")
                idx0 = idxpool.tile([128, JJ], I16, tag="idx0")
                idx1 = idxpool.tile([128, JJ], I16, tag="idx1")
                nc.vector.tensor_scalar(
                    ixw[:], pxw_t[:], 2.0, -1.0, mybir.AluOpType.mult,
                    mybir.AluOpType.add,
                )
                nc.vector.tensor_scalar(
                    ixw[:], ixw[:], 1.0, 99.5, mybir.AluOpType.add,
                    mybir.AluOpType.mult,
                )
                nc.vector.tensor_scalar(x0w[:], ixw[:], -0.5, None, mybir.AluOpType.add)
                nc.vector.tensor_copy(x0i[:], x0w[:])
                nc.vector.tensor_tensor(
                    idx0[:], x0i[:], yoff0_t[:], op=mybir.AluOpType.add
                )
                nc.vector.tensor_tensor(
                    idx1[:], x0i[:], yoff1_t[:], op=mybir.AluOpType.add
                )

                # per-column lerp weights (column-on-partition layout)
                ixc = idxpool.tile([128, GROUPS], F32, tag="ixc")
                x0c = idxpool.tile([128, GROUPS], F32, tag="x0c")
                x0ci = idxpool.tile([128, GROUPS], I16, tag="x0ci")
                fxc = idxpool.tile([128, GROUPS], F32, tag="fxc")
                ufx = idxpool.tile([128, GROUPS], F32, tag="ufx")
                w00 = idxpool.tile([128, GROUPS], F32, tag="w00")
                w01 = idxpool.tile([128, GROUPS], F32, tag="w01")
                w10 = idxpool.tile([128, GROUPS], F32, tag="w10")
                w11 = idxpool.tile([128, GROUPS], F32, tag="w11")
                nc.vector.tensor_scalar(
                    ixc[:], pxc_t[:], 2.0, -1.0, mybir.AluOpType.mult,
                    mybir.AluOpType.add,
                )
                nc.vector.tensor_scalar(
                    ixc[:], ixc[:], 1.0, 99.5, mybir.AluOpType.add,
                    mybir.AluOpType.mult,
                )
                nc.vector.tensor_scalar(x0c[:], ixc[:], -0.5, None, mybir.AluOpType.add)
                nc.vector.tensor_copy(x0ci[:], x0c[:])
                nc.vector.tensor_copy(x0c[:], x0ci[:])
                nc.vector.tensor_tensor(
                    fxc[:], ixc[:], x0c[:], op=mybir.AluOpType.subtract
                )
                nc.vector.tensor_scalar(
                    ufx[:], fxc[:], -1.0, 1.0, mybir.AluOpType.mult, mybir.AluOpType.add
                )
                nc.any.tensor_tensor(w00[:], ufx[:], wy0_t[:], op=mybir.AluOpType.mult)
                nc.any.tensor_tensor(w01[:], fxc[:], wy0_t[:], op=mybir.AluOpType.mult)
                nc.any.tensor_tensor(w10[:], ufx[:], wy1_t[:], op=mybir.AluOpType.mult)
                nc.any.tensor_tensor(w11[:], fxc[:], wy1_t[:], op=mybir.AluOpType.mult)

                table_ap = AP(table, b * HW * C, [[C, HW - 1], [1, 2 * C]])

                for h in range(2):
                    g0 = gpool.tile([128, GH, 2 * C], F32, tag="g0")
                    g1 = gpool.tile([128, GH, 2 * C], F32, tag="g1")
                    nc.gpsimd.dma_gather(
                        g0[:], table_ap, idx0[:, h * (JJ // 2) : (h + 1) * (JJ // 2)],
                        NHALF, NHALF, 2 * C, elem_step=C, single_packet=False,
                    )
                    nc.gpsimd.dma_gather(
                        g1[:], table_ap, idx1[:, h * (JJ // 2) : (h + 1) * (JJ // 2)],
                        NHALF, NHALF, 2 * C, elem_step=C, single_packet=False,
                    )

                    gsl = slice(h * GH, (h + 1) * GH)
                    bshape = [128, GH, C]
                    t0 = lpool.tile([128, GH, C], F32, tag="t0")
                    t1 = lpool.tile([128, GH, C], F32, tag="t1")
                    t2 = lpool.tile([128, GH, C], F32, tag="t2")
                    ot = lpool.tile([128, GH, C], F32, tag="ot")
                    nc.any.tensor_tensor(
                        t0[:], g0[:, :, 0:C], w00[:, gsl].to_broadcast(bshape),
                        op=mybir.AluOpType.mult,
                    )
                    nc.any.tensor_tensor(
                        t1[:], g0[:, :, C : 2 * C], w01[:, gsl].to_broadcast(bshape),
                        op=mybir.AluOpType.mult,
                    )
                    nc.any.tensor_tensor(
                        t2[:], g1[:, :, 0:C], w10[:, gsl].to_broadcast(bshape),
                        op=mybir.AluOpType.mult,
                    )
                    nc.any.tensor_tensor(
                        t0[:], t0[:], t1[:], op=mybir.AluOpType.add
                    )
                    nc.any.tensor_tensor(
                        t1[:], g1[:, :, C : 2 * C], w11[:, gsl].to_broadcast(bshape),
                        op=mybir.AluOpType.mult,
                    )
                    nc.any.tensor_tensor(
                        t2[:], t2[:], t1[:], op=mybir.AluOpType.add
                    )
                    nc.any.tensor_tensor(
                        ot[:], t0[:], t2[:], op=mybir.AluOpType.add
                    )

                    # transpose (cols, ch) -> (ch, cols) on PE, 4 groups per bank
                    otr = opool.tile([C, GH * 128], F32, tag="otr")
                    for g4 in range(0, GH, 4):
                        ng = min(4, GH - g4)
                        ps = pspool.tile([C, 512], F32, tag="ps")
                        for k in range(ng):
                            nc.tensor.transpose(
                                ps[:, k * 128 : (k + 1) * 128],
                                ot[:, g4 + k, :],
                                ident_t[:],
                            )
                        nc.any.tensor_copy(
                            otr[:, g4 * 128 : (g4 + ng) * 128], ps[:, : ng * 128]
                        )

                    # write out: cols are n-major (col = n*S + s)
                    out_ap = AP(
                        out,
                        b * N * C * S + h * (N // 2) * C * S,
                        [[S, C], [C * S, N // 2], [1, S]],
                    )
                    nc.sync.dma_start(
                        out_ap,
                        otr[:].rearrange("c (n s) -> c n s", s=S),
                    )

    nc.compile()
    _nc_cache["nc"] = nc
    return nc


def _prep_core_inputs(feats, px):
    """feats: (BL, C, H, W) f32; px: (BL, N, S) f32 -> input dict."""
    tbl = np.ascontiguousarray(feats.transpose(0, 2, 3, 1)).reshape(-1)
    tbl = np.concatenate([tbl, np.zeros(C, np.float32)])
    pxf = px.reshape(BL, COLS)
    return {
        "table": tbl.astype(np.float32),
        "pxw": np.ascontiguousarray(pxf[:, COLMAP_W]).astype(np.float32),
        "pxc": np.ascontiguousarray(pxf[:, COLMAP_C]).astype(np.float32),
        "yoff0": YOFF0,
        "yoff1": YOFF1,
        "wy0": WY0C,
        "wy1": WY1C,
        "ident": IDENT,
    }


LAST_EXEC_NS = None


def kernel(batch_features, prior_xs):
    global LAST_EXEC_NS
    import os

    batch_features = np.asarray(batch_features, dtype=np.float32)
    prior_xs = np.asarray(prior_xs, dtype=np.float32)
    nc = _build_nc()
    in_maps = [
        _prep_core_inputs(
            batch_features[c * BL : (c + 1) * BL], prior_xs[c * BL : (c + 1) * BL]
        )
        for c in range(NCORES)
    ]
    trace = bool(int(os.environ.get("KERNEL_TRACE", "0")))
    res = run_bass_kernel_spmd(
        nc, in_maps, core_ids=list(range(NCORES)), trace=trace
    )
    if res.exec_time_ns is not None:
        LAST_EXEC_NS = res.exec_time_ns
    outs = [r["out"].reshape(BL * N, C, S, 1) for r in res.results]
    return np.concatenate(outs, axis=0)


if __name__ == "__main__":
    rng = np.random.default_rng(0)
    bf = rng.standard_normal((B, C, H, W), dtype=np.float32)
    px = rng.random((B, N, S), dtype=np.float32)
    o = kernel(bf, px)
    print(o.shape, o.dtype)


# revision 12
# speedup vs baseline: 1.3989x; 1.3989x over previous
"""DetNetV3 pool_prior_features (bilinear grid_sample along lane priors) on 8 trn2 cores.

Strategy (data-parallel over batch, 4 images per core):
- Host: layout-only prep. Features are transposed to NHWC so one 512B DRAM
  element covers both x-taps (x0,x0+1) of all 64 channels. Priors are
  permuted into the two layouts the device needs (gather-index-wrapped and
  column-on-partition). All constant tiles (y-offsets, y-weights, identity)
  are baked host-side; y0/y1/wy are compile-time constants of the module.
- Device: computes x0 = floor(px*199), fx = frac; gathers the 4 taps with
  two dma_gather calls per image half (512B pair elements, y0-row and
  y1-row); applies the 4-tap bilinear weighted sum with tensor_tensor ops
  (weights broadcast over the channel dim with stride-0 APs); transposes
  (cols,ch) -> (ch,cols) on the tensor engine; DMAs to the final
  (B*N, C, S, 1) layout.
"""

import sys

sys.path.insert(0, "/opt/trn_rl_repo")

import numpy as np

import concourse.bass as bass
import concourse.mybir as mybir
from concourse import bacc
from concourse.bass import AP
from concourse.bass_utils import run_bass_kernel_spmd
from concourse.tile import TileContext

F32 = mybir.dt.float32
I16 = mybir.dt.int16

# ---------------------------------------------------------------- constants
B, C, H, W = 32, 64, 80, 200
N, S = 192, 36
NCORES = 8
BL = B // NCORES          # images per core
HW = H * W                # pixels per image
COLS = N * S              # gather columns per image (6912)
NHALF = COLS // 2         # columns per half (3456)
GROUPS = COLS // 128      # 54
GH = GROUPS // 2          # 27 groups per half
JJ = COLS // 16           # idx tile free dim (432)

# y-side constants, computed exactly as the reference does (float32 ops)
_sx = (np.linspace(0.0, 1.0, S, dtype=np.float32) * 71).astype(np.int64)
PRIOR_FEAT_YS = np.ascontiguousarray(
    np.flip(1.0 - _sx.astype(np.float32) / 71)
).astype(np.float32)
_gy = PRIOR_FEAT_YS * np.float32(2.0) - np.float32(1.0)
_iy = (_gy + np.float32(1.0)) * np.float32(0.5) * np.float32(H - 1)
Y0 = np.floor(_iy)
Y1 = Y0 + 1.0
WY1 = (_iy - Y0) * (Y1 <= H - 1)          # mask: zero weight when y1 off-grid
WY0 = 1.0 - (_iy - Y0)
# reference masks the y0 term too (y0 always in [0,79] here, so m=1)
Y0I = Y0.astype(np.int64)
Y1I = np.minimum(Y1, H - 1).astype(np.int64)
WY0 = WY0.astype(np.float32)
WY1 = WY1.astype(np.float32)

# column -> (n, s): col = n*S + s
_cols = np.arange(COLS)
_s_of_col = (_cols % S).astype(np.int64)

# wrapped idx layout: position (q, jj) holds column jj*16 + (q % 16)
_q = np.arange(128)[:, None]
_jj = np.arange(JJ)[None, :]
COLMAP_W = (_jj * 16 + (_q % 16))          # (128, 432)
# column-on-partition layout: position (p, g) holds column g*128 + p
_p = np.arange(128)[:, None]
_g = np.arange(GROUPS)[None, :]
COLMAP_C = (_g * 128 + _p)                 # (128, 54)

SOFF = (_s_of_col * W)[COLMAP_W].astype(np.int16)         # (128, 432)
WY0C = WY0[_s_of_col][COLMAP_C].astype(np.float32)        # (128, 54)
WY1C = WY1[_s_of_col][COLMAP_C].astype(np.float32)
IDENT = np.eye(128, dtype=np.float32)

# 4-tap paired table: entry (s, x) holds [f[y0[s], x, :], f[y1[s], x, :]]
# (2*C floats); a 1KB gather element at entry s*W+x0 covers entries
# (s,x0) and (s,x0+1) == all four bilinear taps of one column.
TBL_LEN = BL * S * W * 2 * C

_nc_cache = {}


def _build_nc():
    if "nc" in _nc_cache:
        return _nc_cache["nc"]
    nc = bacc.Bacc("TRN2")
    table = nc.dram_tensor("table", [TBL_LEN], F32, kind="ExternalInput")
    pxw = nc.dram_tensor("pxw", [BL, 128, JJ], F32, kind="ExternalInput")
    pxc = nc.dram_tensor("pxc", [BL, 128, GROUPS], F32, kind="ExternalInput")
    soff = nc.dram_tensor("soff", [128, JJ], I16, kind="ExternalInput")
    wy0 = nc.dram_tensor("wy0", [128, GROUPS], F32, kind="ExternalInput")
    wy1 = nc.dram_tensor("wy1", [128, GROUPS], F32, kind="ExternalInput")
    ident = nc.dram_tensor("ident", [128, 128], F32, kind="ExternalInput")
    out = nc.dram_tensor("out", [BL * N * C * S], F32, kind="ExternalOutput")

    with TileContext(nc) as tc:
        with (
            tc.tile_pool(name="const", bufs=1) as cpool,
            tc.tile_pool(name="px", bufs=2) as pxpool,
            tc.tile_pool(name="idx", bufs=2) as idxpool,
            tc.tile_pool(name="gath", bufs=2) as gpool,
            tc.tile_pool(name="lerp", bufs=2) as lpool,
            tc.tile_pool(name="outp", bufs=2) as opool,
            tc.tile_pool(name="psum", bufs=4, space="PSUM") as pspool,
        ):
            soff_t = cpool.tile([128, JJ], I16, tag="c0")
            wy0_t = cpool.tile([128, GROUPS], F32, tag="c2")
            wy1_t = cpool.tile([128, GROUPS], F32, tag="c3")
            ident_t = cpool.tile([128, 128], F32, tag="c4")
            nc.sync.dma_start(soff_t[:], soff[:])
            nc.sync.dma_start(wy0_t[:], wy0[:])
            nc.sync.dma_start(wy1_t[:], wy1[:])
            nc.sync.dma_start(ident_t[:], ident[:])

            for b in range(BL):
                pxw_t = pxpool.tile([128, JJ], F32, tag="pxw")
                pxc_t = pxpool.tile([128, GROUPS], F32, tag="pxc")
                nc.sync.dma_start(pxw_t[:], pxw[b])
                nc.sync.dma_start(pxc_t[:], pxc[b])

                # gather indices (wrapped layout): x0 + y*W.
                # ix matches the reference bit-exactly: gx = px*2-1;
                # ix = (gx+1)*0.5*199 == (gx+1)*99.5 (same single rounding).
                # HW f32->int cast is round-half-even, so cast(ix-0.5) is
                # floor(ix) except at odd integers where it yields k-1 with
                # fx=1 -- the lerp result is identical either way.
                ixw = idxpool.tile([128, JJ], F32, tag="ixw")
                x0w = idxpool.tile([128, JJ], F32, tag="x0w")
                x0i = idxpool.tile([128, JJ], I16, tag="x0i")
                idx0 = idxpool.tile([128, JJ], I16, tag="idx0")
                nc.vector.tensor_scalar(
                    ixw[:], pxw_t[:], 2.0, -1.0, mybir.AluOpType.mult,
                    mybir.AluOpType.add,
                )
                nc.vector.tensor_scalar(
                    x0w[:], ixw[:], 1.0, 99.5, mybir.AluOpType.add,
                    mybir.AluOpType.mult,
                )
                nc.vector.tensor_scalar(x0w[:], x0w[:], -0.5, None, mybir.AluOpType.add)
                nc.scalar.copy(x0i[:], x0w[:])
                nc.vector.tensor_tensor(
                    idx0[:], x0i[:], soff_t[:], op=mybir.AluOpType.add
                )

                # per-column lerp weights (column-on-partition layout)
                ixc = idxpool.tile([128, GROUPS], F32, tag="ixc")
                x0c = idxpool.tile([128, GROUPS], F32, tag="x0c")
                x0ci = idxpool.tile([128, GROUPS], I16, tag="x0ci")
                fxc = idxpool.tile([128, GROUPS], F32, tag="fxc")
                ufx = idxpool.tile([128, GROUPS], F32, tag="ufx")
                w00 = idxpool.tile([128, GROUPS], F32, tag="w00")
                w01 = idxpool.tile([128, GROUPS], F32, tag="w01")
                w10 = idxpool.tile([128, GROUPS], F32, tag="w10")
                w11 = idxpool.tile([128, GROUPS], F32, tag="w11")
                nc.vector.tensor_scalar(
                    ixc[:], pxc_t[:], 2.0, -1.0, mybir.AluOpType.mult,
                    mybir.AluOpType.add,
                )
                nc.vector.tensor_scalar(
                    ixc[:], ixc[:], 1.0, 99.5, mybir.AluOpType.add,
                    mybir.AluOpType.mult,
                )
                nc.vector.tensor_scalar(x0c[:], ixc[:], -0.5, None, mybir.AluOpType.add)
                nc.scalar.copy(x0ci[:], x0c[:])
                nc.scalar.copy(x0c[:], x0ci[:])
                nc.vector.tensor_tensor(
                    fxc[:], ixc[:], x0c[:], op=mybir.AluOpType.subtract
                )
                nc.vector.tensor_scalar(
                    ufx[:], fxc[:], -1.0, 1.0, mybir.AluOpType.mult, mybir.AluOpType.add
                )
                nc.any.tensor_tensor(w00[:], ufx[:], wy0_t[:], op=mybir.AluOpType.mult)
                nc.any.tensor_tensor(w01[:], fxc[:], wy0_t[:], op=mybir.AluOpType.mult)
                nc.any.tensor_tensor(w10[:], ufx[:], wy1_t[:], op=mybir.AluOpType.mult)
                nc.any.tensor_tensor(w11[:], fxc[:], wy1_t[:], op=mybir.AluOpType.mult)

                table_ap = AP(table, b * S * W * 2 * C, [[2 * C, S * W - 1], [1, 4 * C]])

                for h in range(2):
                    g0 = gpool.tile([128, GH, 4 * C], F32, tag="g0")
                    nc.gpsimd.dma_gather(
                        g0[:], table_ap, idx0[:, h * (JJ // 2) : (h + 1) * (JJ // 2)],
                        NHALF, NHALF, 4 * C, elem_step=2 * C, single_packet=False,
                    )

                    gsl = slice(h * GH, (h + 1) * GH)
                    bshape = [128, GH, C]
                    t0 = lpool.tile([128, GH, C], F32, tag="t0")
                    t1 = lpool.tile([128, GH, C], F32, tag="t1")
                    t2 = lpool.tile([128, GH, C], F32, tag="t2")
                    ot = lpool.tile([128, GH, C], F32, tag="ot")
                    nc.any.tensor_tensor(
                        t0[:], g0[:, :, 0:C], w00[:, gsl].to_broadcast(bshape),
                        op=mybir.AluOpType.mult,
                    )
                    nc.any.tensor_tensor(
                        t1[:], g0[:, :, 2 * C : 3 * C], w01[:, gsl].to_broadcast(bshape),
                        op=mybir.AluOpType.mult,
                    )
                    nc.any.tensor_tensor(
                        t2[:], g0[:, :, C : 2 * C], w10[:, gsl].to_broadcast(bshape),
                        op=mybir.AluOpType.mult,
                    )
                    nc.any.tensor_tensor(
                        t0[:], t0[:], t1[:], op=mybir.AluOpType.add
                    )
                    nc.any.tensor_tensor(
                        t1[:], g0[:, :, 3 * C : 4 * C], w11[:, gsl].to_broadcast(bshape),
                        op=mybir.AluOpType.mult,
                    )
                    nc.any.tensor_tensor(
                        t2[:], t2[:], t1[:], op=mybir.AluOpType.add
                    )
                    nc.any.tensor_tensor(
                        ot[:], t0[:], t2[:], op=mybir.AluOpType.add
                    )

                    # transpose (cols, ch) -> (ch, cols) on PE, 4 groups per bank
                    otr = opool.tile([C, GH * 128], F32, tag="otr")
                    for g4 in range(0, GH, 4):
                        ng = min(4, GH - g4)
                        ps = pspool.tile([C, 512], F32, tag="ps")
                        for k in range(ng):
                            nc.tensor.transpose(
                                ps[:, k * 128 : (k + 1) * 128],
                                ot[:, g4 + k, :],
                                ident_t[:],
                            )
                        nc.any.tensor_copy(
                            otr[:, g4 * 128 : (g4 + ng) * 128], ps[:, : ng * 128]
                        )

                    # write out: cols are n-major (col = n*S + s)
                    out_ap = AP(
                        out,
                        b * N * C * S + h * (N // 2) * C * S,
                        [[S, C], [C * S, N // 2], [1, S]],
                    )
                    nc.sync.dma_start(
                        out_ap,
                        otr[:].rearrange("c (n s) -> c n s", s=S),
                    )

    nc.compile()
    _nc_cache["nc"] = nc
    return nc


def _prep_core_inputs(feats, px):
    """feats: (BL, C, H, W) f32; px: (BL, N, S) f32 -> input dict."""
    nhwc = feats.transpose(0, 2, 3, 1)                      # (BL, H, W, C)
    t4 = np.empty((BL, S, W, 2, C), np.float32)
    t4[:, :, :, 0, :] = nhwc[:, Y0I, :, :]
    t4[:, :, :, 1, :] = nhwc[:, Y1I, :, :]
    pxf = px.reshape(BL, COLS)
    return {
        "table": t4.reshape(-1),
        "pxw": np.ascontiguousarray(pxf[:, COLMAP_W]).astype(np.float32),
        "pxc": np.ascontiguousarray(pxf[:, COLMAP_C]).astype(np.float32),
        "soff": SOFF,
        "wy0": WY0C,
        "wy1": WY1C,
        "ident": IDENT,
    }


LAST_EXEC_NS = None


def kernel(batch_features, prior_xs):
    global LAST_EXEC_NS
    import os

    batch_features = np.asarray(batch_features, dtype=np.float32)
    prior_xs = np.asarray(prior_xs, dtype=np.float32)
    nc = _build_nc()
    in_maps = [
        _prep_core_inputs(
            batch_features[c * BL : (c + 1) * BL], prior_xs[c * BL : (c + 1) * BL]
        )
        for c in range(NCORES)
    ]
    trace = bool(int(os.environ.get("KERNEL_TRACE", "0")))
    res = run_bass_kernel_spmd(
        nc, in_maps, core_ids=list(range(NCORES)), trace=trace
    )
    if res.exec_time_ns is not None:
        LAST_EXEC_NS = res.exec_time_ns
    outs = [r["out"].reshape(BL * N, C, S, 1) for r in res.results]
    return np.concatenate(outs, axis=0)


if __name__ == "__main__":
    rng = np.random.default_rng(0)
    bf = rng.standard_normal((B, C, H, W), dtype=np.float32)
    px = rng.random((B, N, S), dtype=np.float32)
    o = kernel(bf, px)
    print(o.shape, o.dtype)


# revision 15
# speedup vs baseline: 1.5348x; 1.0972x over previous
"""DetNetV3 pool_prior_features (bilinear grid_sample along lane priors) on 8 trn2 cores.

Strategy (data-parallel over batch, 4 images per core):
- Host: layout-only prep. Features are transposed to NHWC so one 512B DRAM
  element covers both x-taps (x0,x0+1) of all 64 channels. Priors are
  permuted into the two layouts the device needs (gather-index-wrapped and
  column-on-partition). All constant tiles (y-offsets, y-weights, identity)
  are baked host-side; y0/y1/wy are compile-time constants of the module.
- Device: computes x0 = floor(px*199), fx = frac; gathers the 4 taps with
  two dma_gather calls per image half (512B pair elements, y0-row and
  y1-row); applies the 4-tap bilinear weighted sum with tensor_tensor ops
  (weights broadcast over the channel dim with stride-0 APs); transposes
  (cols,ch) -> (ch,cols) on the tensor engine; DMAs to the final
  (B*N, C, S, 1) layout.
"""

import sys

sys.path.insert(0, "/opt/trn_rl_repo")

import numpy as np

import concourse.bass as bass
import concourse.mybir as mybir
from concourse import bacc
from concourse.bass import AP
from concourse.bass_utils import run_bass_kernel_spmd
from concourse.tile import TileContext

F32 = mybir.dt.float32
I16 = mybir.dt.int16

# ---------------------------------------------------------------- constants
B, C, H, W = 32, 64, 80, 200
N, S = 192, 36
NCORES = 8
BL = B // NCORES          # images per core
HW = H * W                # pixels per image
COLS = N * S              # gather columns per image (6912)
NHALF = COLS // 2         # columns per half (3456)
GROUPS = COLS // 128      # 54
GH = GROUPS // 2          # 27 groups per half
JJ = COLS // 16           # idx tile free dim (432)

# y-side constants, computed exactly as the reference does (float32 ops)
_sx = (np.linspace(0.0, 1.0, S, dtype=np.float32) * 71).astype(np.int64)
PRIOR_FEAT_YS = np.ascontiguousarray(
    np.flip(1.0 - _sx.astype(np.float32) / 71)
).astype(np.float32)
_gy = PRIOR_FEAT_YS * np.float32(2.0) - np.float32(1.0)
_iy = (_gy + np.float32(1.0)) * np.float32(0.5) * np.float32(H - 1)
Y0 = np.floor(_iy)
Y1 = Y0 + 1.0
WY1 = (_iy - Y0) * (Y1 <= H - 1)          # mask: zero weight when y1 off-grid
WY0 = 1.0 - (_iy - Y0)
# reference masks the y0 term too (y0 always in [0,79] here, so m=1)
Y0I = Y0.astype(np.int64)
Y1I = np.minimum(Y1, H - 1).astype(np.int64)
WY0 = WY0.astype(np.float32)
WY1 = WY1.astype(np.float32)

# column -> (n, s): col = n*S + s
_cols = np.arange(COLS)
_s_of_col = (_cols % S).astype(np.int64)

# wrapped idx layout: position (q, jj) holds column jj*16 + (q % 16)
_q = np.arange(128)[:, None]
_jj = np.arange(JJ)[None, :]
COLMAP_W = (_jj * 16 + (_q % 16))          # (128, 432)
# column-on-partition layout: position (p, g) holds column g*128 + p
_p = np.arange(128)[:, None]
_g = np.arange(GROUPS)[None, :]
COLMAP_C = (_g * 128 + _p)                 # (128, 54)

SOFF = (_s_of_col * W)[COLMAP_W].astype(np.int16)         # (128, 432)
WY0C = WY0[_s_of_col][COLMAP_C].astype(np.float32)        # (128, 54)
WY1C = WY1[_s_of_col][COLMAP_C].astype(np.float32)
IDENT = np.eye(128, dtype=np.float32)

# 4-tap paired table: entry (s, x) holds [f[y0[s], x, :], f[y1[s], x, :]]
# (2*C floats); a 1KB gather element at entry s*W+x0 covers entries
# (s,x0) and (s,x0+1) == all four bilinear taps of one column.
TBL_LEN = BL * S * W * 2 * C

_nc_cache = {}


def _build_nc():
    if "nc" in _nc_cache:
        return _nc_cache["nc"]
    nc = bacc.Bacc("TRN2")
    table = nc.dram_tensor("table", [TBL_LEN], F32, kind="ExternalInput")
    pxw = nc.dram_tensor("pxw", [BL, 128, JJ], F32, kind="ExternalInput")
    pxc = nc.dram_tensor("pxc", [BL, 128, GROUPS], F32, kind="ExternalInput")
    soff = nc.dram_tensor("soff", [128, JJ], I16, kind="ExternalInput")
    wy0 = nc.dram_tensor("wy0", [128, GROUPS], F32, kind="ExternalInput")
    wy1 = nc.dram_tensor("wy1", [128, GROUPS], F32, kind="ExternalInput")
    ident = nc.dram_tensor("ident", [128, 128], F32, kind="ExternalInput")
    out = nc.dram_tensor("out", [BL * N * C * S], F32, kind="ExternalOutput")

    with TileContext(nc) as tc:
        with (
            tc.tile_pool(name="const", bufs=1) as cpool,
            tc.tile_pool(name="px", bufs=2) as pxpool,
            tc.tile_pool(name="idx", bufs=2) as idxpool,
            tc.tile_pool(name="gath", bufs=3) as gpool,
            tc.tile_pool(name="lerp", bufs=2) as lpool,
            tc.tile_pool(name="outp", bufs=2) as opool,
            tc.tile_pool(name="psum", bufs=4, space="PSUM") as pspool,
        ):
            soff_t = cpool.tile([128, JJ], I16, tag="c0")
            wy0_t = cpool.tile([128, GROUPS], F32, tag="c2")
            wy1_t = cpool.tile([128, GROUPS], F32, tag="c3")
            ident_t = cpool.tile([128, 128], F32, tag="c4")
            nc.sync.dma_start(soff_t[:], soff[:])
            nc.sync.dma_start(wy0_t[:], wy0[:])
            nc.sync.dma_start(wy1_t[:], wy1[:])
            nc.sync.dma_start(ident_t[:], ident[:])

            for b in range(BL):
                pxw_t = pxpool.tile([128, JJ], F32, tag="pxw")
                pxc_t = pxpool.tile([128, GROUPS], F32, tag="pxc")
                nc.sync.dma_start(pxw_t[:], pxw[b])
                nc.sync.dma_start(pxc_t[:], pxc[b])

                # gather indices (wrapped layout): x0 + y*W.
                # ix matches the reference bit-exactly: gx = px*2-1;
                # ix = (gx+1)*0.5*199 == (gx+1)*99.5 (same single rounding).
                # HW f32->int cast is round-half-even, so cast(ix-0.5) is
                # floor(ix) except at odd integers where it yields k-1 with
                # fx=1 -- the lerp result is identical either way.
                ixw = idxpool.tile([128, JJ], F32, tag="ixw")
                x0w = idxpool.tile([128, JJ], F32, tag="x0w")
                x0i = idxpool.tile([128, JJ], I16, tag="x0i")
                idx0 = idxpool.tile([128, JJ], I16, tag="idx0")
                nc.vector.tensor_scalar(
                    ixw[:], pxw_t[:], 2.0, -1.0, mybir.AluOpType.mult,
                    mybir.AluOpType.add,
                )
                nc.vector.tensor_scalar(
                    x0w[:], ixw[:], 1.0, 99.5, mybir.AluOpType.add,
                    mybir.AluOpType.mult,
                )
                nc.vector.tensor_scalar(x0w[:], x0w[:], -0.5, None, mybir.AluOpType.add)
                nc.scalar.copy(x0i[:], x0w[:])
                nc.vector.tensor_tensor(
                    idx0[:], x0i[:], soff_t[:], op=mybir.AluOpType.add
                )

                # per-column lerp weights (column-on-partition layout)
                ixc = idxpool.tile([128, GROUPS], F32, tag="ixc")
                x0c = idxpool.tile([128, GROUPS], F32, tag="x0c")
                x0ci = idxpool.tile([128, GROUPS], I16, tag="x0ci")
                fxc = idxpool.tile([128, GROUPS], F32, tag="fxc")
                ufx = idxpool.tile([128, GROUPS], F32, tag="ufx")
                w00 = idxpool.tile([128, GROUPS], F32, tag="w00")
                w01 = idxpool.tile([128, GROUPS], F32, tag="w01")
                w10 = idxpool.tile([128, GROUPS], F32, tag="w10")
                w11 = idxpool.tile([128, GROUPS], F32, tag="w11")
                nc.vector.tensor_scalar(
                    ixc[:], pxc_t[:], 2.0, -1.0, mybir.AluOpType.mult,
                    mybir.AluOpType.add,
                )
                nc.vector.tensor_scalar(
                    ixc[:], ixc[:], 1.0, 99.5, mybir.AluOpType.add,
                    mybir.AluOpType.mult,
                )
                nc.vector.tensor_scalar(x0c[:], ixc[:], -0.5, None, mybir.AluOpType.add)
                nc.scalar.copy(x0ci[:], x0c[:])
                nc.scalar.copy(x0c[:], x0ci[:])
                nc.vector.tensor_tensor(
                    fxc[:], ixc[:], x0c[:], op=mybir.AluOpType.subtract
                )
                nc.vector.tensor_scalar(
                    ufx[:], fxc[:], -1.0, 1.0, mybir.AluOpType.mult, mybir.AluOpType.add
                )
                nc.any.tensor_tensor(w00[:], ufx[:], wy0_t[:], op=mybir.AluOpType.mult)
                nc.any.tensor_tensor(w01[:], fxc[:], wy0_t[:], op=mybir.AluOpType.mult)
                nc.any.tensor_tensor(w10[:], ufx[:], wy1_t[:], op=mybir.AluOpType.mult)
                nc.any.tensor_tensor(w11[:], fxc[:], wy1_t[:], op=mybir.AluOpType.mult)

                table_ap = AP(table, b * S * W * 2 * C, [[2 * C, S * W - 1], [1, 4 * C]])

                for h in range(2):
                    g0 = gpool.tile([128, GH, 4 * C], F32, tag="g0")
                    nc.gpsimd.dma_gather(
                        g0[:], table_ap, idx0[:, h * (JJ // 2) : (h + 1) * (JJ // 2)],
                        NHALF, NHALF, 4 * C, elem_step=2 * C, single_packet=False,
                    )

                    gsl = slice(h * GH, (h + 1) * GH)
                    bshape = [128, GH, C]
                    t0 = lpool.tile([128, GH, C], F32, tag="t0")
                    t1 = lpool.tile([128, GH, C], F32, tag="t1")
                    t2 = lpool.tile([128, GH, C], F32, tag="t2")
                    ot = lpool.tile([128, GH, C], F32, tag="ot")
                    nc.any.tensor_tensor(
                        t0[:], g0[:, :, 0:C], w00[:, gsl].to_broadcast(bshape),
                        op=mybir.AluOpType.mult,
                    )
                    nc.any.tensor_tensor(
                        t1[:], g0[:, :, 2 * C : 3 * C], w01[:, gsl].to_broadcast(bshape),
                        op=mybir.AluOpType.mult,
                    )
                    nc.any.tensor_tensor(
                        t2[:], g0[:, :, C : 2 * C], w10[:, gsl].to_broadcast(bshape),
                        op=mybir.AluOpType.mult,
                    )
                    nc.any.tensor_tensor(
                        t0[:], t0[:], t1[:], op=mybir.AluOpType.add
                    )
                    nc.any.tensor_tensor(
                        t1[:], g0[:, :, 3 * C : 4 * C], w11[:, gsl].to_broadcast(bshape),
                        op=mybir.AluOpType.mult,
                    )
                    nc.any.tensor_tensor(
                        t2[:], t2[:], t1[:], op=mybir.AluOpType.add
                    )
                    nc.any.tensor_tensor(
                        ot[:], t0[:], t2[:], op=mybir.AluOpType.add
                    )

                    # transpose (cols, ch) -> (ch, cols) on PE, 4 groups per bank
                    otr = opool.tile([C, GH * 128], F32, tag="otr")
                    for g4 in range(0, GH, 4):
                        ng = min(4, GH - g4)
                        ps = pspool.tile([C, 512], F32, tag="ps")
                        for k in range(ng):
                            nc.tensor.transpose(
                                ps[:, k * 128 : (k + 1) * 128],
                                ot[:, g4 + k, :],
                                ident_t[:],
                            )
                        nc.any.tensor_copy(
                            otr[:, g4 * 128 : (g4 + ng) * 128], ps[:, : ng * 128]
                        )

                    # write out: cols are n-major (col = n*S + s)
                    out_ap = AP(
                        out,
                        b * N * C * S + h * (N // 2) * C * S,
                        [[S, C], [C * S, N // 2], [1, S]],
                    )
                    nc.sync.dma_start(
                        out_ap,
                        otr[:].rearrange("c (n s) -> c n s", s=S),
                    )

    nc.compile()
    _nc_cache["nc"] = nc
    return nc


def _prep_core_inputs(feats, px):
    """feats: (BL, C, H, W) f32; px: (BL, N, S) f32 -> input dict."""
    nhwc = feats.transpose(0, 2, 3, 1)                      # (BL, H, W, C)
    t4 = np.empty((BL, S, W, 2, C), np.float32)
    t4[:, :, :, 0, :] = nhwc[:, Y0I, :, :]
    t4[:, :, :, 1, :] = nhwc[:, Y1I, :, :]
    pxf = px.reshape(BL, COLS)
    return {
        "table": t4.reshape(-1),
        "pxw": np.ascontiguousarray(pxf[:, COLMAP_W]).astype(np.float32),
        "pxc": np.ascontiguousarray(pxf[:, COLMAP_C]).astype(np.float32),
        "soff": SOFF,
        "wy0": WY0C,
        "wy1": WY1C,
        "ident": IDENT,
    }


LAST_EXEC_NS = None


def kernel(batch_features, prior_xs):
    global LAST_EXEC_NS
    import os

    batch_features = np.asarray(batch_features, dtype=np.float32)
    prior_xs = np.asarray(prior_xs, dtype=np.float32)
    nc = _build_nc()
    in_maps = [
        _prep_core_inputs(
            batch_features[c * BL : (c + 1) * BL], prior_xs[c * BL : (c + 1) * BL]
        )
        for c in range(NCORES)
    ]
    trace = bool(int(os.environ.get("KERNEL_TRACE", "0")))
    res = run_bass_kernel_spmd(
        nc, in_maps, core_ids=list(range(NCORES)), trace=trace
    )
    if res.exec_time_ns is not None:
        LAST_EXEC_NS = res.exec_time_ns
    outs = [r["out"].reshape(BL * N, C, S, 1) for r in res.results]
    return np.concatenate(outs, axis=0)


if __name__ == "__main__":
    rng = np.random.default_rng(0)
    bf = rng.standard_normal((B, C, H, W), dtype=np.float32)
    px = rng.random((B, N, S), dtype=np.float32)
    o = kernel(bf, px)
    print(o.shape, o.dtype)


# revision 16
# speedup vs baseline: 1.6729x; 1.0900x over previous
"""DetNetV3 pool_prior_features (bilinear grid_sample along lane priors) on 8 trn2 cores.

Strategy (data-parallel over batch, 4 images per core):
- Host: layout-only prep. Features are transposed to NHWC so one 512B DRAM
  element covers both x-taps (x0,x0+1) of all 64 channels. Priors are
  permuted into the two layouts the device needs (gather-index-wrapped and
  column-on-partition). All constant tiles (y-offsets, y-weights, identity)
  are baked host-side; y0/y1/wy are compile-time constants of the module.
- Device: computes x0 = floor(px*199), fx = frac; gathers the 4 taps with
  two dma_gather calls per image half (512B pair elements, y0-row and
  y1-row); applies the 4-tap bilinear weighted sum with tensor_tensor ops
  (weights broadcast over the channel dim with stride-0 APs); transposes
  (cols,ch) -> (ch,cols) on the tensor engine; DMAs to the final
  (B*N, C, S, 1) layout.
"""

import sys

sys.path.insert(0, "/opt/trn_rl_repo")

import numpy as np

import concourse.bass as bass
import concourse.mybir as mybir
from concourse import bacc
from concourse.bass import AP
from concourse.bass_utils import run_bass_kernel_spmd
from concourse.tile import TileContext

F32 = mybir.dt.float32
I16 = mybir.dt.int16

# ---------------------------------------------------------------- constants
B, C, H, W = 32, 64, 80, 200
N, S = 192, 36
NCORES = 8
BL = B // NCORES          # images per core
HW = H * W                # pixels per image
COLS = N * S              # gather columns per image (6912)
NHALF = COLS // 2         # columns per half (3456)
GROUPS = COLS // 128      # 54
GH = GROUPS // 2          # 27 groups per half
JJ = COLS // 16           # idx tile free dim (432)

# y-side constants, computed exactly as the reference does (float32 ops)
_sx = (np.linspace(0.0, 1.0, S, dtype=np.float32) * 71).astype(np.int64)
PRIOR_FEAT_YS = np.ascontiguousarray(
    np.flip(1.0 - _sx.astype(np.float32) / 71)
).astype(np.float32)
_gy = PRIOR_FEAT_YS * np.float32(2.0) - np.float32(1.0)
_iy = (_gy + np.float32(1.0)) * np.float32(0.5) * np.float32(H - 1)
Y0 = np.floor(_iy)
Y1 = Y0 + 1.0
WY1 = (_iy - Y0) * (Y1 <= H - 1)          # mask: zero weight when y1 off-grid
WY0 = 1.0 - (_iy - Y0)
# reference masks the y0 term too (y0 always in [0,79] here, so m=1)
Y0I = Y0.astype(np.int64)
Y1I = np.minimum(Y1, H - 1).astype(np.int64)
WY0 = WY0.astype(np.float32)
WY1 = WY1.astype(np.float32)

# column -> (n, s): col = n*S + s
_cols = np.arange(COLS)
_s_of_col = (_cols % S).astype(np.int64)

# wrapped idx layout: position (q, jj) holds column jj*16 + (q % 16)
_q = np.arange(128)[:, None]
_jj = np.arange(JJ)[None, :]
COLMAP_W = (_jj * 16 + (_q % 16))          # (128, 432)
# column-on-partition layout: position (p, g) holds column g*128 + p
_p = np.arange(128)[:, None]
_g = np.arange(GROUPS)[None, :]
COLMAP_C = (_g * 128 + _p)                 # (128, 54)

SOFF = (_s_of_col * W)[COLMAP_W].astype(np.int16)         # (128, 432)
WY0C = WY0[_s_of_col][COLMAP_C].astype(np.float32)        # (128, 54)
WY1C = WY1[_s_of_col][COLMAP_C].astype(np.float32)
IDENT = np.eye(128, dtype=np.float32)

# 4-tap paired table: entry (s, x) holds [f[y0[s], x, :], f[y1[s], x, :]]
# (2*C floats); a 1KB gather element at entry s*W+x0 covers entries
# (s,x0) and (s,x0+1) == all four bilinear taps of one column.
TBL_LEN = BL * S * W * 2 * C

_nc_cache = {}


def _build_nc():
    if "nc" in _nc_cache:
        return _nc_cache["nc"]
    nc = bacc.Bacc("TRN2")
    table = nc.dram_tensor("table", [TBL_LEN], F32, kind="ExternalInput")
    pxw = nc.dram_tensor("pxw", [BL, 128, JJ], F32, kind="ExternalInput")
    pxc = nc.dram_tensor("pxc", [BL, 128, GROUPS], F32, kind="ExternalInput")
    soff = nc.dram_tensor("soff", [128, JJ], I16, kind="ExternalInput")
    wy0 = nc.dram_tensor("wy0", [128, GROUPS], F32, kind="ExternalInput")
    wy1 = nc.dram_tensor("wy1", [128, GROUPS], F32, kind="ExternalInput")
    ident = nc.dram_tensor("ident", [128, 128], F32, kind="ExternalInput")
    out = nc.dram_tensor("out", [BL * N * C * S], F32, kind="ExternalOutput")

    with TileContext(nc) as tc:
        with (
            tc.tile_pool(name="const", bufs=1) as cpool,
            tc.tile_pool(name="px", bufs=2) as pxpool,
            tc.tile_pool(name="idx", bufs=2) as idxpool,
            tc.tile_pool(name="gath", bufs=3) as gpool,
            tc.tile_pool(name="lerp", bufs=2) as lpool,
            tc.tile_pool(name="outp", bufs=2) as opool,
            tc.tile_pool(name="psum", bufs=4, space="PSUM") as pspool,
        ):
            soff_t = cpool.tile([128, JJ], I16, tag="c0")
            wy0_t = cpool.tile([128, GROUPS], F32, tag="c2")
            wy1_t = cpool.tile([128, GROUPS], F32, tag="c3")
            ident_t = cpool.tile([128, 128], F32, tag="c4")
            nc.sync.dma_start(soff_t[:], soff[:])
            nc.sync.dma_start(wy0_t[:], wy0[:])
            nc.sync.dma_start(wy1_t[:], wy1[:])
            nc.sync.dma_start(ident_t[:], ident[:])

            idx0_l, w_l = [], []
            for b in range(BL):
                pxw_t = pxpool.tile([128, JJ], F32, tag="pxw")
                pxc_t = pxpool.tile([128, GROUPS], F32, tag="pxc")
                nc.sync.dma_start(pxw_t[:], pxw[b])
                nc.sync.dma_start(pxc_t[:], pxc[b])

                # gather indices (wrapped layout): x0 + y*W.
                # ix matches the reference bit-exactly: gx = px*2-1;
                # ix = (gx+1)*0.5*199 == (gx+1)*99.5 (same single rounding).
                # HW f32->int cast is round-half-even, so cast(ix-0.5) is
                # floor(ix) except at odd integers where it yields k-1 with
                # fx=1 -- the lerp result is identical either way.
                ixw = idxpool.tile([128, JJ], F32, tag="ixw")
                x0w = idxpool.tile([128, JJ], F32, tag="x0w")
                x0i = idxpool.tile([128, JJ], I16, tag="x0i")
                idx0 = idxpool.tile([128, JJ], I16, tag=f"idx0_{b}")
                nc.vector.tensor_scalar(
                    ixw[:], pxw_t[:], 2.0, -1.0, mybir.AluOpType.mult,
                    mybir.AluOpType.add,
                )
                nc.vector.tensor_scalar(
                    x0w[:], ixw[:], 1.0, 99.5, mybir.AluOpType.add,
                    mybir.AluOpType.mult,
                )
                nc.vector.tensor_scalar(x0w[:], x0w[:], -0.5, None, mybir.AluOpType.add)
                nc.scalar.copy(x0i[:], x0w[:])
                nc.vector.tensor_tensor(
                    idx0[:], x0i[:], soff_t[:], op=mybir.AluOpType.add
                )

                # per-column lerp weights (column-on-partition layout)
                ixc = idxpool.tile([128, GROUPS], F32, tag="ixc")
                x0c = idxpool.tile([128, GROUPS], F32, tag="x0c")
                x0ci = idxpool.tile([128, GROUPS], I16, tag="x0ci")
                fxc = idxpool.tile([128, GROUPS], F32, tag="fxc")
                ufx = idxpool.tile([128, GROUPS], F32, tag="ufx")
                w00 = idxpool.tile([128, GROUPS], F32, tag=f"w00_{b}")
                w01 = idxpool.tile([128, GROUPS], F32, tag=f"w01_{b}")
                w10 = idxpool.tile([128, GROUPS], F32, tag=f"w10_{b}")
                w11 = idxpool.tile([128, GROUPS], F32, tag=f"w11_{b}")
                nc.vector.tensor_scalar(
                    ixc[:], pxc_t[:], 2.0, -1.0, mybir.AluOpType.mult,
                    mybir.AluOpType.add,
                )
                nc.vector.tensor_scalar(
                    ixc[:], ixc[:], 1.0, 99.5, mybir.AluOpType.add,
                    mybir.AluOpType.mult,
                )
                nc.vector.tensor_scalar(x0c[:], ixc[:], -0.5, None, mybir.AluOpType.add)
                nc.scalar.copy(x0ci[:], x0c[:])
                nc.scalar.copy(x0c[:], x0ci[:])
                nc.vector.tensor_tensor(
                    fxc[:], ixc[:], x0c[:], op=mybir.AluOpType.subtract
                )
                nc.vector.tensor_scalar(
                    ufx[:], fxc[:], -1.0, 1.0, mybir.AluOpType.mult, mybir.AluOpType.add
                )
                nc.any.tensor_tensor(w00[:], ufx[:], wy0_t[:], op=mybir.AluOpType.mult)
                nc.any.tensor_tensor(w01[:], fxc[:], wy0_t[:], op=mybir.AluOpType.mult)
                nc.any.tensor_tensor(w10[:], ufx[:], wy1_t[:], op=mybir.AluOpType.mult)
                nc.any.tensor_tensor(w11[:], fxc[:], wy1_t[:], op=mybir.AluOpType.mult)
                idx0_l.append(idx0)
                w_l.append((w00, w01, w10, w11))

            for b in range(BL):
                idx0 = idx0_l[b]
                w00, w01, w10, w11 = w_l[b]
                table_ap = AP(table, b * S * W * 2 * C, [[2 * C, S * W - 1], [1, 4 * C]])

                for h in range(2):
                    g0 = gpool.tile([128, GH, 4 * C], F32, tag="g0")
                    nc.gpsimd.dma_gather(
                        g0[:], table_ap, idx0[:, h * (JJ // 2) : (h + 1) * (JJ // 2)],
                        NHALF, NHALF, 4 * C, elem_step=2 * C, single_packet=False,
                    )

                    gsl = slice(h * GH, (h + 1) * GH)
                    bshape = [128, GH, C]
                    t0 = lpool.tile([128, GH, C], F32, tag="t0")
                    t1 = lpool.tile([128, GH, C], F32, tag="t1")
                    t2 = lpool.tile([128, GH, C], F32, tag="t2")
                    ot = lpool.tile([128, GH, C], F32, tag="ot")
                    nc.any.tensor_tensor(
                        t0[:], g0[:, :, 0:C], w00[:, gsl].to_broadcast(bshape),
                        op=mybir.AluOpType.mult,
                    )
                    nc.any.tensor_tensor(
                        t1[:], g0[:, :, 2 * C : 3 * C], w01[:, gsl].to_broadcast(bshape),
                        op=mybir.AluOpType.mult,
                    )
                    nc.any.tensor_tensor(
                        t2[:], g0[:, :, C : 2 * C], w10[:, gsl].to_broadcast(bshape),
                        op=mybir.AluOpType.mult,
                    )
                    nc.any.tensor_tensor(
                        t0[:], t0[:], t1[:], op=mybir.AluOpType.add
                    )
                    nc.any.tensor_tensor(
                        t1[:], g0[:, :, 3 * C : 4 * C], w11[:, gsl].to_broadcast(bshape),
                        op=mybir.AluOpType.mult,
                    )
                    nc.any.tensor_tensor(
                        t2[:], t2[:], t1[:], op=mybir.AluOpType.add
                    )
                    nc.any.tensor_tensor(
                        ot[:], t0[:], t2[:], op=mybir.AluOpType.add
                    )

                    # transpose (cols, ch) -> (ch, cols) on PE, 4 groups per bank
                    otr = opool.tile([C, GH * 128], F32, tag="otr")
                    for g4 in range(0, GH, 4):
                        ng = min(4, GH - g4)
                        ps = pspool.tile([C, 512], F32, tag="ps")
                        for k in range(ng):
                            nc.tensor.transpose(
                                ps[:, k * 128 : (k + 1) * 128],
                                ot[:, g4 + k, :],
                                ident_t[:],
                            )
                        nc.any.tensor_copy(
                            otr[:, g4 * 128 : (g4 + ng) * 128], ps[:, : ng * 128]
                        )

                    # write out: cols are n-major (col = n*S + s)
                    out_ap = AP(
                        out,
                        b * N * C * S + h * (N // 2) * C * S,
                        [[S, C], [C * S, N // 2], [1, S]],
                    )
                    nc.sync.dma_start(
                        out_ap,
                        otr[:].rearrange("c (n s) -> c n s", s=S),
                    )

    nc.compile()
    _nc_cache["nc"] = nc
    return nc


def _prep_core_inputs(feats, px):
    """feats: (BL, C, H, W) f32; px: (BL, N, S) f32 -> input dict."""
    nhwc = feats.transpose(0, 2, 3, 1)                      # (BL, H, W, C)
    t4 = np.empty((BL, S, W, 2, C), np.float32)
    t4[:, :, :, 0, :] = nhwc[:, Y0I, :, :]
    t4[:, :, :, 1, :] = nhwc[:, Y1I, :, :]
    pxf = px.reshape(BL, COLS)
    return {
        "table": t4.reshape(-1),
        "pxw": np.ascontiguousarray(pxf[:, COLMAP_W]).astype(np.float32),
        "pxc": np.ascontiguousarray(pxf[:, COLMAP_C]).astype(np.float32),
        "soff": SOFF,
        "wy0": WY0C,
        "wy1": WY1C,
        "ident": IDENT,
    }


LAST_EXEC_NS = None


def kernel(batch_features, prior_xs):
    global LAST_EXEC_NS
    import os

    batch_features = np.asarray(batch_features, dtype=np.float32)
    prior_xs = np.asarray(prior_xs, dtype=np.float32)
    nc = _build_nc()
    in_maps = [
        _prep_core_inputs(
            batch_features[c * BL : (c + 1) * BL], prior_xs[c * BL : (c + 1) * BL]
        )
        for c in range(NCORES)
    ]
    trace = bool(int(os.environ.get("KERNEL_TRACE", "0")))
    res = run_bass_kernel_spmd(
        nc, in_maps, core_ids=list(range(NCORES)), trace=trace
    )
    if res.exec_time_ns is not None:
        LAST_EXEC_NS = res.exec_time_ns
    outs = [r["out"].reshape(BL * N, C, S, 1) for r in res.results]
    return np.concatenate(outs, axis=0)


if __name__ == "__main__":
    rng = np.random.default_rng(0)
    bf = rng.standard_normal((B, C, H, W), dtype=np.float32)
    px = rng.random((B, N, S), dtype=np.float32)
    o = kernel(bf, px)
    print(o.shape, o.dtype)


# revision 17
# speedup vs baseline: 1.9545x; 1.1683x over previous
"""DetNetV3 pool_prior_features (bilinear grid_sample along lane priors) on 8 trn2 cores.

Strategy (data-parallel over batch, 4 images per core):
- Host: layout-only prep. Features are transposed to NHWC so one 512B DRAM
  element covers both x-taps (x0,x0+1) of all 64 channels. Priors are
  permuted into the two layouts the device needs (gather-index-wrapped and
  column-on-partition). All constant tiles (y-offsets, y-weights, identity)
  are baked host-side; y0/y1/wy are compile-time constants of the module.
- Device: computes x0 = floor(px*199), fx = frac; gathers the 4 taps with
  two dma_gather calls per image half (512B pair elements, y0-row and
  y1-row); applies the 4-tap bilinear weighted sum with tensor_tensor ops
  (weights broadcast over the channel dim with stride-0 APs); transposes
  (cols,ch) -> (ch,cols) on the tensor engine; DMAs to the final
  (B*N, C, S, 1) layout.
"""

import sys

sys.path.insert(0, "/opt/trn_rl_repo")

import numpy as np

import concourse.bass as bass
import concourse.mybir as mybir
from concourse import bacc
from concourse.bass import AP
from concourse.bass_utils import run_bass_kernel_spmd
from concourse.tile import TileContext

F32 = mybir.dt.float32
I16 = mybir.dt.int16

# ---------------------------------------------------------------- constants
B, C, H, W = 32, 64, 80, 200
N, S = 192, 36
NCORES = 8
BL = B // NCORES          # images per core
HW = H * W                # pixels per image
COLS = N * S              # gather columns per image (6912)
NHALF = COLS // 2         # columns per half (3456)
GROUPS = COLS // 128      # 54
GH = GROUPS // 2          # 27 groups per half
JJ = COLS // 16           # idx tile free dim (432)

# y-side constants, computed exactly as the reference does (float32 ops)
_sx = (np.linspace(0.0, 1.0, S, dtype=np.float32) * 71).astype(np.int64)
PRIOR_FEAT_YS = np.ascontiguousarray(
    np.flip(1.0 - _sx.astype(np.float32) / 71)
).astype(np.float32)
_gy = PRIOR_FEAT_YS * np.float32(2.0) - np.float32(1.0)
_iy = (_gy + np.float32(1.0)) * np.float32(0.5) * np.float32(H - 1)
Y0 = np.floor(_iy)
Y1 = Y0 + 1.0
WY1 = (_iy - Y0) * (Y1 <= H - 1)          # mask: zero weight when y1 off-grid
WY0 = 1.0 - (_iy - Y0)
# reference masks the y0 term too (y0 always in [0,79] here, so m=1)
Y0I = Y0.astype(np.int64)
Y1I = np.minimum(Y1, H - 1).astype(np.int64)
WY0 = WY0.astype(np.float32)
WY1 = WY1.astype(np.float32)

# column -> (n, s): col = n*S + s
_cols = np.arange(COLS)
_s_of_col = (_cols % S).astype(np.int64)

# wrapped idx layout: position (q, jj) holds column jj*16 + (q % 16)
_q = np.arange(128)[:, None]
_jj = np.arange(JJ)[None, :]
COLMAP_W = (_jj * 16 + (_q % 16))          # (128, 432)
# column-on-partition layout: position (p, g) holds column g*128 + p
_p = np.arange(128)[:, None]
_g = np.arange(GROUPS)[None, :]
COLMAP_C = (_g * 128 + _p)                 # (128, 54)

SOFF = (_s_of_col * W)[COLMAP_W].astype(np.int16)         # (128, 432)
WY0C = WY0[_s_of_col][COLMAP_C].astype(np.float32)        # (128, 54)
WY1C = WY1[_s_of_col][COLMAP_C].astype(np.float32)
IDENT = np.eye(128, dtype=np.float32)

# 4-tap paired table: entry (s, x) holds [f[y0[s], x, :], f[y1[s], x, :]]
# (2*C floats); a 1KB gather element at entry s*W+x0 covers entries
# (s,x0) and (s,x0+1) == all four bilinear taps of one column.
TBL_LEN = BL * S * W * 2 * C

_nc_cache = {}


def _build_nc():
    if "nc" in _nc_cache:
        return _nc_cache["nc"]
    nc = bacc.Bacc("TRN2")
    table = nc.dram_tensor("table", [TBL_LEN], F32, kind="ExternalInput")
    pxw = nc.dram_tensor("pxw", [BL, 128, JJ], F32, kind="ExternalInput")
    pxc = nc.dram_tensor("pxc", [BL, 128, GROUPS], F32, kind="ExternalInput")
    soff = nc.dram_tensor("soff", [128, JJ], I16, kind="ExternalInput")
    wy0 = nc.dram_tensor("wy0", [128, GROUPS], F32, kind="ExternalInput")
    wy1 = nc.dram_tensor("wy1", [128, GROUPS], F32, kind="ExternalInput")
    ident = nc.dram_tensor("ident", [128, 128], F32, kind="ExternalInput")
    out = nc.dram_tensor("out", [BL * N * C * S], F32, kind="ExternalOutput")

    with TileContext(nc) as tc:
        with (
            tc.tile_pool(name="const", bufs=1) as cpool,
            tc.tile_pool(name="px", bufs=2) as pxpool,
            tc.tile_pool(name="idx", bufs=2) as idxpool,
            tc.tile_pool(name="gath", bufs=4) as gpool,
            tc.tile_pool(name="lerp", bufs=3) as lpool,
            tc.tile_pool(name="outp", bufs=2) as opool,
            tc.tile_pool(name="psum", bufs=4, space="PSUM") as pspool,
        ):
            soff_t = cpool.tile([128, JJ], I16, tag="c0")
            wy0_t = cpool.tile([128, GROUPS], F32, tag="c2")
            wy1_t = cpool.tile([128, GROUPS], F32, tag="c3")
            ident_t = cpool.tile([128, 128], F32, tag="c4")
            nc.sync.dma_start(soff_t[:], soff[:])
            nc.sync.dma_start(wy0_t[:], wy0[:])
            nc.sync.dma_start(wy1_t[:], wy1[:])
            nc.sync.dma_start(ident_t[:], ident[:])

            idx0_l, w_l = [], []
            for b in range(BL):
                pxw_t = pxpool.tile([128, JJ], F32, tag="pxw")
                pxc_t = pxpool.tile([128, GROUPS], F32, tag="pxc")
                nc.sync.dma_start(pxw_t[:], pxw[b])
                nc.sync.dma_start(pxc_t[:], pxc[b])

                # gather indices (wrapped layout): x0 + y*W.
                # ix matches the reference bit-exactly: gx = px*2-1;
                # ix = (gx+1)*0.5*199 == (gx+1)*99.5 (same single rounding).
                # HW f32->int cast is round-half-even, so cast(ix-0.5) is
                # floor(ix) except at odd integers where it yields k-1 with
                # fx=1 -- the lerp result is identical either way.
                ixw = idxpool.tile([128, JJ], F32, tag="ixw")
                x0w = idxpool.tile([128, JJ], F32, tag="x0w")
                x0i = idxpool.tile([128, JJ], I16, tag="x0i")
                idx0 = idxpool.tile([128, JJ], I16, tag=f"idx0_{b}")
                nc.vector.tensor_scalar(
                    ixw[:], pxw_t[:], 2.0, -1.0, mybir.AluOpType.mult,
                    mybir.AluOpType.add,
                )
                nc.vector.tensor_scalar(
                    x0w[:], ixw[:], 1.0, 99.5, mybir.AluOpType.add,
                    mybir.AluOpType.mult,
                )
                nc.vector.tensor_scalar(x0w[:], x0w[:], -0.5, None, mybir.AluOpType.add)
                nc.scalar.copy(x0i[:], x0w[:])
                nc.vector.tensor_tensor(
                    idx0[:], x0i[:], soff_t[:], op=mybir.AluOpType.add
                )

                # per-column lerp weights (column-on-partition layout)
                ixc = idxpool.tile([128, GROUPS], F32, tag="ixc")
                x0c = idxpool.tile([128, GROUPS], F32, tag="x0c")
                x0ci = idxpool.tile([128, GROUPS], I16, tag="x0ci")
                fxc = idxpool.tile([128, GROUPS], F32, tag="fxc")
                ufx = idxpool.tile([128, GROUPS], F32, tag="ufx")
                w00 = idxpool.tile([128, GROUPS], F32, tag=f"w00_{b}")
                w01 = idxpool.tile([128, GROUPS], F32, tag=f"w01_{b}")
                w10 = idxpool.tile([128, GROUPS], F32, tag=f"w10_{b}")
                w11 = idxpool.tile([128, GROUPS], F32, tag=f"w11_{b}")
                nc.vector.tensor_scalar(
                    ixc[:], pxc_t[:], 2.0, -1.0, mybir.AluOpType.mult,
                    mybir.AluOpType.add,
                )
                nc.vector.tensor_scalar(
                    ixc[:], ixc[:], 1.0, 99.5, mybir.AluOpType.add,
                    mybir.AluOpType.mult,
                )
                nc.vector.tensor_scalar(x0c[:], ixc[:], -0.5, None, mybir.AluOpType.add)
                nc.scalar.copy(x0ci[:], x0c[:])
                nc.scalar.copy(x0c[:], x0ci[:])
                nc.vector.tensor_tensor(
                    fxc[:], ixc[:], x0c[:], op=mybir.AluOpType.subtract
                )
                nc.vector.tensor_scalar(
                    ufx[:], fxc[:], -1.0, 1.0, mybir.AluOpType.mult, mybir.AluOpType.add
                )
                nc.any.tensor_tensor(w00[:], ufx[:], wy0_t[:], op=mybir.AluOpType.mult)
                nc.any.tensor_tensor(w01[:], fxc[:], wy0_t[:], op=mybir.AluOpType.mult)
                nc.any.tensor_tensor(w10[:], ufx[:], wy1_t[:], op=mybir.AluOpType.mult)
                nc.any.tensor_tensor(w11[:], fxc[:], wy1_t[:], op=mybir.AluOpType.mult)
                idx0_l.append(idx0)
                w_l.append((w00, w01, w10, w11))

            for b in range(BL):
                idx0 = idx0_l[b]
                w00, w01, w10, w11 = w_l[b]
                table_ap = AP(table, b * S * W * 2 * C, [[2 * C, S * W - 1], [1, 4 * C]])

                for h in range(6):
                    NP, GP, JP = COLS // 6, GROUPS // 6, JJ // 6
                    g0 = gpool.tile([128, GP, 4 * C], F32, tag="g0")
                    nc.gpsimd.dma_gather(
                        g0[:], table_ap, idx0[:, h * JP : (h + 1) * JP],
                        NP, NP, 4 * C, elem_step=2 * C, single_packet=False,
                    )

                    gsl = slice(h * GP, (h + 1) * GP)
                    bshape = [128, GP, C]
                    t0 = lpool.tile([128, GP, C], F32, tag="t0")
                    t1 = lpool.tile([128, GP, C], F32, tag="t1")
                    t2 = lpool.tile([128, GP, C], F32, tag="t2")
                    ot = lpool.tile([128, GP, C], F32, tag="ot")
                    nc.any.tensor_tensor(
                        t0[:], g0[:, :, 0:C], w00[:, gsl].to_broadcast(bshape),
                        op=mybir.AluOpType.mult,
                    )
                    nc.any.tensor_tensor(
                        t1[:], g0[:, :, 2 * C : 3 * C], w01[:, gsl].to_broadcast(bshape),
                        op=mybir.AluOpType.mult,
                    )
                    nc.any.tensor_tensor(
                        t2[:], g0[:, :, C : 2 * C], w10[:, gsl].to_broadcast(bshape),
                        op=mybir.AluOpType.mult,
                    )
                    nc.any.tensor_tensor(
                        t0[:], t0[:], t1[:], op=mybir.AluOpType.add
                    )
                    nc.any.tensor_tensor(
                        t1[:], g0[:, :, 3 * C : 4 * C], w11[:, gsl].to_broadcast(bshape),
                        op=mybir.AluOpType.mult,
                    )
                    nc.any.tensor_tensor(
                        t2[:], t2[:], t1[:], op=mybir.AluOpType.add
                    )
                    nc.any.tensor_tensor(
                        ot[:], t0[:], t2[:], op=mybir.AluOpType.add
                    )

                    # transpose (cols, ch) -> (ch, cols) on PE, 4 groups per bank
                    otr = opool.tile([C, GP * 128], F32, tag="otr")
                    for g4 in range(0, GP, 4):
                        ng = min(4, GP - g4)
                        ps = pspool.tile([C, 512], F32, tag="ps")
                        for k in range(ng):
                            nc.tensor.transpose(
                                ps[:, k * 128 : (k + 1) * 128],
                                ot[:, g4 + k, :],
                                ident_t[:],
                            )
                        nc.any.tensor_copy(
                            otr[:, g4 * 128 : (g4 + ng) * 128], ps[:, : ng * 128]
                        )

                    # write out: cols are n-major (col = n*S + s)
                    out_ap = AP(
                        out,
                        b * N * C * S + h * (N // 6) * C * S,
                        [[S, C], [C * S, N // 6], [1, S]],
                    )
                    nc.sync.dma_start(
                        out_ap,
                        otr[:].rearrange("c (n s) -> c n s", s=S),
                    )

    nc.compile()
    _nc_cache["nc"] = nc
    return nc


def _prep_core_inputs(feats, px):
    """feats: (BL, C, H, W) f32; px: (BL, N, S) f32 -> input dict."""
    nhwc = feats.transpose(0, 2, 3, 1)                      # (BL, H, W, C)
    t4 = np.empty((BL, S, W, 2, C), np.float32)
    t4[:, :, :, 0, :] = nhwc[:, Y0I, :, :]
    t4[:, :, :, 1, :] = nhwc[:, Y1I, :, :]
    pxf = px.reshape(BL, COLS)
    return {
        "table": t4.reshape(-1),
        "pxw": np.ascontiguousarray(pxf[:, COLMAP_W]).astype(np.float32),
        "pxc": np.ascontiguousarray(pxf[:, COLMAP_C]).astype(np.float32),
        "soff": SOFF,
        "wy0": WY0C,
        "wy1": WY1C,
        "ident": IDENT,
    }


LAST_EXEC_NS = None


def kernel(batch_features, prior_xs):
    global LAST_EXEC_NS
    import os

    batch_features = np.asarray(batch_features, dtype=np.float32)
    prior_xs = np.asarray(prior_xs, dtype=np.float32)
    nc = _build_nc()
    in_maps = [
        _prep_core_inputs(
            batch_features[c * BL : (c + 1) * BL], prior_xs[c * BL : (c + 1) * BL]
        )
        for c in range(NCORES)
    ]
    trace = bool(int(os.environ.get("KERNEL_TRACE", "0")))
    res = run_bass_kernel_spmd(
        nc, in_maps, core_ids=list(range(NCORES)), trace=trace
    )
    if res.exec_time_ns is not None:
        LAST_EXEC_NS = res.exec_time_ns
    outs = [r["out"].reshape(BL * N, C, S, 1) for r in res.results]
    return np.concatenate(outs, axis=0)


if __name__ == "__main__":
    rng = np.random.default_rng(0)
    bf = rng.standard_normal((B, C, H, W), dtype=np.float32)
    px = rng.random((B, N, S), dtype=np.float32)
    o = kernel(bf, px)
    print(o.shape, o.dtype)
